# revision 1
# baseline (speedup 1.0000x reference)
"""Trainium2 Bass kernel for masked-softmax attention scoring.

Reference computation (B=128, T=512, K=1024, Q=1024):
    mids  = einsum("kq,bq->bk", W, query)
    s     = tanh(einsum("btk,bk->bt", key, mids) + bias)
    attn  = softmax-like: exp(s - max) * mask / sum(exp(s - max) * mask)

The max-subtraction cancels exactly in the ratio (tanh is bounded), so the
device computes  attn = exp(tanh(.)) * mask / sum_t(exp(tanh(.)) * mask).

Sharding: data-parallel over B across 8 NeuronCores (16 batches/core).
Per-core layout: partition p = (b, j) with b in [0,16), j in [0,8);
free column c in [0,64); timestep t = j*64 + c.

The mids matmul writes the (b, j)-replicated layout directly: the
stationary operand is query^T with each batch column replicated 8x via a
stride-0 DVE copy, fed as float32r (full-rate fp32 on the PE).  The W^T
prologue is split half-and-half across the two HWDGE FIFO rings so it
lands at aggregate HBM bandwidth; key chunks follow on both rings with
half-size chunks at the head and tail, consumed by 64 fused
multiply-reduce DVE ops (affine_mul_reduce) in merged arrival order.
Softmax normalization does the 8-partition group sum with a
block-diagonal 0/1 matmul.
"""

import sys

if "/opt/trn_rl_repo" not in sys.path:
    sys.path.insert(0, "/opt/trn_rl_repo")

from contextlib import ExitStack

import numpy as np

# ---- problem constants (hardcoded per spec) ----
B, T, K, Q = 128, 512, 1024, 1024
NCORES = 8
BS = B // NCORES          # 16 batches per core
P = 128                   # SBUF partitions
J = P // BS               # 8 t-blocks per batch on partitions
CF = T // J               # 64 timesteps per (partition, free col)
CC = 4                    # t-cols per key DMA super-chunk (2 MB each)
NCH = CF // CC            # 16 key DMAs per core
QC = Q // P               # 8 contraction chunks for the mids matmul
KEY_BUFS = 7              # key tile pool depth

_STATE: dict = {}


def _build_nc():
    import concourse.tile as tile
    from concourse import bacc, mybir

    f32 = mybir.dt.float32
    f32r = mybir.dt.float32r
    nc = bacc.Bacc()

    qt_e = nc.declare_dram_parameter("qt", [P, QC, BS], f32r, isOutput=False)
    wt_e = nc.declare_dram_parameter("wt", [P, QC, K], f32r, isOutput=False)
    grp_e = nc.declare_dram_parameter("grp", [P, P], f32, isOutput=False)
    key_e = nc.declare_dram_parameter("key", [BS, T, K], f32, isOutput=False)
    maskr_e = nc.declare_dram_parameter("maskr", [P, CF], f32, isOutput=False)
    bias_e = nc.declare_dram_parameter("biasb", [P, 1], f32, isOutput=False)
    out_e = nc.declare_dram_parameter("out", [P, CF], f32, isOutput=True)

    with tile.TileContext(nc) as tc, ExitStack() as ctx:
        const = ctx.enter_context(tc.tile_pool(name="const", bufs=1))
        kpool = ctx.enter_context(tc.tile_pool(name="key", bufs=KEY_BUFS))
        spool = ctx.enter_context(tc.tile_pool(name="scratch", bufs=2))
        psum = ctx.enter_context(tc.tile_pool(name="psum", bufs=1, space="PSUM"))

        # ---- prologue loads split across BOTH HWDGE rings (~2.1 MB each)
        # so W^T lands at full aggregate bandwidth (~20 us) and neither
        # ring idles before the key stream.
        qt_sb = const.tile([P, QC, BS], f32r)
        nc.sync.dma_start(out=qt_sb[:], in_=qt_e[:])
        wt_sb = const.tile([P, QC, K], f32r)
        for qc in range(QC // 2):
            nc.sync.dma_start(out=wt_sb[:, qc, :], in_=wt_e[:, qc, :])
        for qc in range(QC // 2, QC):
            nc.scalar.dma_start(out=wt_sb[:, qc, :], in_=wt_e[:, qc, :])
        grp_sb = const.tile([P, P], f32)
        nc.sync.dma_start(out=grp_sb[:], in_=grp_e[:])
        maskr_sb = const.tile([P, CF], f32)
        nc.sync.dma_start(out=maskr_sb[:], in_=maskr_e[:])
        bias_sb = const.tile([P, 1], f32)
        nc.sync.dma_start(out=bias_sb[:], in_=bias_e[:])

        # ---- mids in broadcast layout: [P, K], row p = mids[b(p), :] ----
        # Replicate each query column 8x on-chip (stride-0 DVE read) so the
        # stationary operand has the (b, j) partition order in one free dim.
        qtrep_sb = const.tile([P, QC, BS, J], f32r)
        nc.vector.tensor_copy(
            qtrep_sb[:], qt_sb[:].unsqueeze(-1).broadcast_to((P, QC, BS, J))
        )
        # matmuls in wt-chunk ARRIVAL order (rings deliver 0..3 and 4..7
        # concurrently); PSUM accumulation order is numerically immaterial.
        mids_ps = psum.tile([P, K], f32)
        qc_order = [0, 4, 1, 5, 2, 6, 3, 7]
        for qi, qc in enumerate(qc_order):
            lhsT = qtrep_sb[:, qc, :, :]
            for h in range(2):
                nc.tensor.matmul(
                    mids_ps[:, h * 512 : (h + 1) * 512],
                    lhsT=lhsT,
                    rhs=wt_sb[:, qc, h * 512 : (h + 1) * 512],
                    start=(qi == 0),
                    stop=(qi == QC - 1),
                )
        mids_bc = const.tile([P, K], f32)
        nc.vector.tensor_copy(mids_bc[:], mids_ps[:])

        # ---- scores[p, c] = key[b, j*64+c, :] . mids[b, :] ----
        # Both HWDGE FIFO rings stream 32 score-columns of key each, behind
        # their ~2.1 MB W^T halves; the final chunks are half-size so the
        # post-last-arrival DVE tail is short.  The DVE consumes chunks in
        # merged arrival order (model: equal per-ring column cadence).
        # Small chunks at the head (tolerate early arrival-order noise
        # cheaply) and at the tail (short post-last-arrival DVE tail).
        RING_COLS = {
            "A": [2, 2, 2, 2, 4, 4, 4, 4, 4, 2, 2],   # 32 cols
            "B": [4, 4, 4, 4, 4, 4, 4, 2, 2],          # 32 cols
        }
        entries = []
        for ring, pro in (("A", 6.8), ("B", 6.3)):
            t = pro
            for k, sz in enumerate(RING_COLS[ring]):
                t += sz * 2.9
                entries.append((t, ring, k, sz))
        entries.sort()
        scores_sb = const.tile([P, CF], f32)
        key_r = key_e[:].rearrange("b (j c) k -> (b j) c k", j=J)
        ring_eng = {"A": nc.sync, "B": nc.scalar}
        sched = []
        base = 0
        for t, ring, k, sz in entries:
            sched.append((ring, sz, base))
            base += sz
        for ring, sz, c0 in sched:
            kt = kpool.tile([P, CC, K], f32, tag="ktile")
            ring_eng[ring].dma_start(
                out=kt[:, 0:sz, :], in_=key_r[:, c0 : c0 + sz, :]
            )
            for cc in range(sz):
                c = c0 + cc
                prod = spool.tile([P, K], f32, tag="prod")
                nc.vector.affine_mul_reduce(
                    out=prod[:],
                    accum_out=scores_sb[:, c : c + 1],
                    in0=kt[:, cc, :],
                    in1=mids_bc[:],
                    scale=1.0,
                    bias=0.0,
                )

        # ---- epilogue: tanh, exp, mask, normalize ----
        tanh_sb = const.tile([P, CF], f32)
        nc.scalar.activation(
            out=tanh_sb[:],
            in_=scores_sb[:],
            func=mybir.ActivationFunctionType.Tanh,
            bias=bias_sb[:],
            scale=1.0,
        )
        exp_sb = const.tile([P, CF], f32)
        nc.scalar.activation(
            out=exp_sb[:], in_=tanh_sb[:], func=mybir.ActivationFunctionType.Exp
        )
        em_sb = const.tile([P, CF], f32)
        rowsum = const.tile([P, 1], f32)
        nc.vector.affine_mul_reduce(
            out=em_sb[:],
            accum_out=rowsum[:],
            in0=exp_sb[:],
            in1=maskr_sb[:],
            scale=1.0,
            bias=0.0,
        )
        den_ps = psum.tile([P, 1], f32)
        nc.tensor.matmul(
            den_ps[:], lhsT=grp_sb[:], rhs=rowsum[:], start=True, stop=True
        )
        rinv = const.tile([P, 1], f32)
        nc.vector.reciprocal(out=rinv[:], in_=den_ps[:])
        attn_sb = const.tile([P, CF], f32)
        nc.vector.tensor_scalar_mul(attn_sb[:], em_sb[:], rinv[:])
        nc.scalar.dma_start(out=out_e[:], in_=attn_sb[:])

    nc.compile()
    return nc


def _get_nc():
    if "nc" not in _STATE:
        _STATE["nc"] = _build_nc()
    return _STATE["nc"]


def _grp():
    if "GRP" not in _STATE:
        # GRP[p, m] = 1 iff p // J == m // J  (block-diagonal group-sum)
        pj = np.arange(P) // J
        _STATE["GRP"] = np.ascontiguousarray(
            (pj[:, None] == pj[None, :]).astype(np.float32)
        )
    return _STATE["GRP"]


def _make_in_maps(query, key, mask, W, bias):
    query = np.asarray(query, dtype=np.float32)
    key = np.asarray(key, dtype=np.float32)
    mask = np.asarray(mask, dtype=np.float32)
    W = np.asarray(W, dtype=np.float32)
    bias = np.asarray(bias, dtype=np.float32).reshape(-1)

    # wt[p, qc, k] = W.T[qc*128 + p, k]
    WT = np.ascontiguousarray(
        np.ascontiguousarray(W.T).reshape(QC, P, K).transpose(1, 0, 2)
    )
    GRP = _grp()
    biasb = np.ascontiguousarray(
        np.broadcast_to(bias[:1][None, :], (P, 1)).astype(np.float32)
    )

    in_maps = []
    for i in range(NCORES):
        sh = slice(i * BS, (i + 1) * BS)
        in_maps.append(
            {
                # pre-laid [P, QC, BS]: qt[p, qc, b] = query[sh].T[qc*128+p, b]
                "qt": np.ascontiguousarray(
                    query[sh].T.reshape(QC, P, BS).transpose(1, 0, 2)
                ),
                "wt": WT,
                "grp": GRP,
                "key": np.ascontiguousarray(key[sh]),
                "maskr": np.ascontiguousarray(mask[sh]).reshape(P, CF),
                "biasb": biasb,
            }
        )
    return in_maps


def _run(in_maps, **kwargs):
    from concourse.bass_utils import run_bass_kernel_spmd

    return run_bass_kernel_spmd(
        _get_nc(), in_maps, core_ids=list(range(NCORES)), **kwargs
    )


def _gather(results):
    return np.concatenate(
        [np.asarray(r["out"]).reshape(BS, T) for r in results], axis=0
    )


def kernel(query, key, mask, W, bias):
    in_maps = _make_in_maps(query, key, mask, W, bias)
    res = _run(in_maps)
    return _gather(res.results)



# revision 8
# speedup vs baseline: 1.6480x; 1.6480x over previous
"""Trainium2 Bass kernel for masked-softmax attention scoring.

Reference computation (B=128, T=512, K=1024, Q=1024):
    mids  = einsum("kq,bq->bk", W, query)
    s     = tanh(einsum("btk,bk->bt", key, mids) + bias)
    attn  = softmax-like: exp(s - max) * mask / sum(exp(s - max) * mask)

The max-subtraction cancels exactly in the ratio (tanh is bounded), so the
device computes  attn = exp(tanh(.)) * mask / sum_t(exp(tanh(.)) * mask).

Sharding: data-parallel over B across 8 NeuronCores (16 batches/core).

v2 strategy (vs the DVE fp32 baseline at ~117us):
  * All large operands ship as fp16 (key 32MB->16MB, W^T 4MB->2MB per
    core), halving the HBM-DMA roofline.  Scores pass through a heavily
    saturated tanh (|score| std ~59), so fp16 rounding perturbs the final
    attention weights by rel_l2 ~1.4e-3 -- far inside the 2e-2 gate.
  * The per-batch dot products scores[b,t] = key[b,t,:].mids[b,:] run on
    the TENSOR engine (1 col/cycle at fp16 = ~27us for the 64K streamed
    columns, hidden under the ~45us key DMA) instead of the DVE, whose
    1x custom-op rate (~1.04ns/elem) would otherwise become the new
    bottleneck at ~73us.  The stationary is mids^T [128k x 16b]; each
    matmul streams a key chunk [128k x 512t]; out [16, 512] accumulates
    over the 8 k-chunks in a PSUM bank and only row b=w is kept.
  * Key is host-transposed to [p=k%128, w, kc, t] fp16 so each window's
    chunk is one contiguous-per-partition 1MB DMA; 16 chunks alternate
    across both HWDGE rings behind the W^T/query prologue.  The whole
    key fits in SBUF, so every DMA issues immediately (no recycling).
  * Per window: one Act op fuses the PSUM drain with tanh(+bias) into
    the [16,512] score tile; bulk epilogue does exp, mask-mul+row-sum
    (DVE affine_mul_reduce), reciprocal, scale, and the output DMA.
"""

import sys

if "/opt/trn_rl_repo" not in sys.path:
    sys.path.insert(0, "/opt/trn_rl_repo")

from contextlib import ExitStack

import numpy as np

# ---- problem constants (hardcoded per spec) ----
B, T, K, Q = 128, 512, 1024, 1024
NCORES = 8
BS = B // NCORES          # 16 batches per core (= windows)
P = 128                   # SBUF partitions
KC = K // P               # 8 contraction chunks for the score matmuls
QC = Q // P               # 8 contraction chunks for the mids matmul
WIN_BUFS = 4              # PSUM window-accumulator pool depth

_STATE: dict = {}


def _build_nc():
    import concourse.tile as tile
    from concourse import bacc, mybir

    f32 = mybir.dt.float32
    f16 = mybir.dt.float16
    nc = bacc.Bacc()

    qt_e = nc.declare_dram_parameter("qt", [P, QC, BS], f16, isOutput=False)
    wt_e = nc.declare_dram_parameter("wt", [P, KC, QC, P], f16, isOutput=False)
    key_e = nc.declare_dram_parameter("keyt", [P, BS, KC, T], f16, isOutput=False)
    maskr_e = nc.declare_dram_parameter("maskr", [BS, T], f32, isOutput=False)
    bias_e = nc.declare_dram_parameter("biasb", [BS, 1], f32, isOutput=False)
    eye_e = nc.declare_dram_parameter("eye", [BS, BS], f32, isOutput=False)
    out_e = nc.declare_dram_parameter("out", [BS, T], f32, isOutput=True)

    with tile.TileContext(nc) as tc, ExitStack() as ctx:
        const = ctx.enter_context(tc.tile_pool(name="const", bufs=1))
        psum = ctx.enter_context(tc.tile_pool(name="psum", bufs=1, space="PSUM"))
        wpool = ctx.enter_context(
            tc.tile_pool(name="win", bufs=WIN_BUFS, space="PSUM")
        )

        # ---- prologue loads split across BOTH HWDGE rings ----
        # ring A (sync/SP): W^T halves; ring B (scalar/Act): query, the
        # other W^T half, mask, bias.  ~1.1MB per ring before key starts.
        qt_sb = const.tile([P, QC, BS], f16)
        nc.scalar.dma_start(out=qt_sb[:], in_=qt_e[:])
        wt_sb = const.tile([P, KC, QC, P], f16)
        nc.sync.dma_start(out=wt_sb[:, 0:4, :, :], in_=wt_e[:, 0:4, :, :])
        nc.scalar.dma_start(out=wt_sb[:, 4:8, :, :], in_=wt_e[:, 4:8, :, :])
        maskr_sb = const.tile([BS, T], f32)
        nc.scalar.dma_start(out=maskr_sb[:], in_=maskr_e[:])
        bias_sb = const.tile([BS, 1], f32)
        nc.scalar.dma_start(out=bias_sb[:], in_=bias_e[:])
        eye_sb = const.tile([BS, BS], f32)
        nc.scalar.dma_start(out=eye_sb[:], in_=eye_e[:])

        # ---- mids^T tiles: midsT[kt][p, b] = mids[b, kt*128+p] ----
        # mids[b, k] = sum_q W[k, q] query[b, q]; accumulate over the 8
        # q-chunks with W^T chunks stationary, then cast fp32->fp16 so the
        # tiles can serve as fp16 stationaries for the score matmuls.
        midsT_ps = psum.tile([P, KC, BS], f32)
        midsT_sb = const.tile([P, KC, BS], f16)
        for kt in range(KC):
            for qc in range(QC):
                nc.tensor.matmul(
                    midsT_ps[:, kt, :],
                    lhsT=wt_sb[:, kt, qc, :],
                    rhs=qt_sb[:, qc, :],
                    start=(qc == 0),
                    stop=(qc == QC - 1),
                )
            nc.vector.tensor_copy(midsT_sb[:, kt, :], midsT_ps[:, kt, :])

        # ---- scores: stream key windows through the PE ----
        # window w == batch b; chunk w is [128, KC, T] fp16 (1MB), rings
        # alternate.  Per chunk: 8 accumulating matmuls into a PSUM bank
        # out[b', t] = sum_k mids[b', k] key[w, t, k]; only row b'=w is
        # real.  Engines cannot address single partitions (quadrant rule),
        # so the Act engine tanh's the FULL [16, T] tile out of PSUM and a
        # DVE affine_then_add with a one-hot per-partition scale (eye
        # column w) accumulates row w into a ping-pong assembly of
        # tanh(scores + bias); garbage rows are scaled by 0.
        key_sb = const.tile([P, BS, KC, T], f16)
        ring = {0: nc.sync, 1: nc.scalar}
        dpool = ctx.enter_context(tc.tile_pool(name="drain", bufs=3))
        acc_a = const.tile([BS, T], f32)
        acc_b = const.tile([BS, T], f32)
        acc = [acc_a, acc_b]
        for w in range(BS):
            ring[w % 2].dma_start(
                out=key_sb[:, w, :, :], in_=key_e[:, w, :, :]
            )
        for w in range(BS):
            win = wpool.tile([P, T], f32, tag="win")
            for kc in range(KC):
                nc.tensor.matmul(
                    win[0:BS, :],
                    lhsT=midsT_sb[:, kc, :],
                    rhs=key_sb[:, w, kc, :],
                    start=(kc == 0),
                    stop=(kc == KC - 1),
                )
            drain = dpool.tile([BS, T], f32, tag="drain")
            nc.scalar.activation(
                out=drain[:],
                in_=win[0:BS, :],
                func=mybir.ActivationFunctionType.Tanh,
                bias=bias_sb[:],
                scale=1.0,
            )
            if w == 0:
                nc.vector.tensor_scalar_mul(
                    acc[0][:], drain[:], eye_sb[:, 0:1]
                )
            else:
                nc.vector.affine_then_add(
                    out=acc[w % 2][:],
                    in0=drain[:],
                    in1=acc[(w - 1) % 2][:],
                    scale=eye_sb[:, w : w + 1],
                    bias=0.0,
                )
        tanh_sb = acc[(BS - 1) % 2]

        # ---- epilogue: exp, mask, normalize ----
        exp_sb = const.tile([BS, T], f32)
        nc.scalar.activation(
            out=exp_sb[:], in_=tanh_sb[:], func=mybir.ActivationFunctionType.Exp
        )
        em_sb = const.tile([BS, T], f32)
        rowsum = const.tile([BS, 1], f32)
        nc.vector.affine_mul_reduce(
            out=em_sb[:],
            accum_out=rowsum[:],
            in0=exp_sb[:],
            in1=maskr_sb[:],
            scale=1.0,
            bias=0.0,
        )
        rinv = const.tile([BS, 1], f32)
        nc.vector.reciprocal(out=rinv[:], in_=rowsum[:])
        attn_sb = const.tile([BS, T], f32)
        nc.vector.tensor_scalar_mul(attn_sb[:], em_sb[:], rinv[:])
        nc.sync.dma_start(out=out_e[:], in_=attn_sb[:])

    nc.compile()
    return nc


def _get_nc():
    if "nc" not in _STATE:
        _STATE["nc"] = _build_nc()
    return _STATE["nc"]


def _make_in_maps(query, key, mask, W, bias):
    query = np.asarray(query, dtype=np.float32)
    key = np.asarray(key, dtype=np.float32)
    mask = np.asarray(mask, dtype=np.float32)
    W = np.asarray(W, dtype=np.float32)
    bias = np.asarray(bias, dtype=np.float32).reshape(-1)

    # wt[p, kt, qc, k'] = W[kt*128+k', qc*128+p]
    WT = np.ascontiguousarray(
        W.reshape(KC, P, QC, P).transpose(3, 0, 2, 1).astype(np.float16)
    )
    biasb = np.ascontiguousarray(
        np.broadcast_to(bias[:1][None, :], (BS, 1)).astype(np.float32)
    )
    eye = np.ascontiguousarray(np.eye(BS, dtype=np.float32))

    in_maps = []
    for i in range(NCORES):
        sh = slice(i * BS, (i + 1) * BS)
        # keyt[p, w, kc, t] = key[b0+w, t, kc*128+p]
        kt = np.ascontiguousarray(
            key[sh].reshape(BS, T, KC, P).transpose(3, 0, 2, 1).astype(np.float16)
        )
        # qt[p, qc, b] = query[b0+b, qc*128+p]
        qt = np.ascontiguousarray(
            query[sh].reshape(BS, QC, P).transpose(2, 1, 0).astype(np.float16)
        )
        in_maps.append(
            {
                "qt": qt,
                "wt": WT,
                "keyt": kt,
                "maskr": np.ascontiguousarray(mask[sh]),
                "biasb": biasb,
                "eye": eye,
            }
        )
    return in_maps


def _run(in_maps, **kwargs):
    from concourse.bass_utils import run_bass_kernel_spmd

    return run_bass_kernel_spmd(
        _get_nc(), in_maps, core_ids=list(range(NCORES)), **kwargs
    )


def _gather(results):
    return np.concatenate(
        [np.asarray(r["out"]).reshape(BS, T) for r in results], axis=0
    )


def kernel(query, key, mask, W, bias):
    in_maps = _make_in_maps(query, key, mask, W, bias)
    res = _run(in_maps)
    return _gather(res.results)
